# revision 5
# baseline (speedup 1.0000x reference)
"""Trainium2 Bass kernel for batched attention with softmax over the query axis.

Reference computation (per batch element b):
    Q = tokens @ Wq; K = tokens @ Wk; V = tokens @ Wv
    S = Q @ K.T                [T(t), T(s)]
    A = softmax(S, axis=t)     (normalizes over the *query* axis per key column)
    out = A @ V                [T, H]

Sharding: pure data parallelism - B=8 batch elements, one per NeuronCore.
The softmax couples queries only within a batch element, so no collectives.

Per-core implementation (fp16 matmul operands, fp32 PSUM accumulation):
  - The PE does ONLY matmuls (no PE transposes): all transposes go through
    the DMA xbar (dma_start(..., transpose=True), fp16, one instruction per
    [128, 512] tile with a 3D [128, 4, 128] destination AP). The xbars ride
    the scalar HWDGE ring so they are not FIFO-queued behind the bulk input
    descriptors on the sync ring.
  - W_qk = Wq @ Wk.T built on-chip, so scores need one projection
    GT = Wqk^T @ tok^T instead of separate Q and K: S = tokT^T @ GT.
  - tokens: per-tile DMA -> GpSimd f32->f16 cast -> xbar transpose -> tokT.
  - S tile [s%128, t] accumulates into one 4-bank PSUM tile [128, 2048];
    softmax over t (free axis): one DVE reduce_max (negated), one ScalarE
    exp over all 2048 columns with accum_out row sums.
  - 1/rowsum folded into V rows (DVE), unnormalized exp E feeds ctx matmul.
  - Warm-up junk matmuls at t=0 keep the PE HAM clock at K=8/8 through the
    initial DMA window; a dummy exp preloads the ACT table set early.
Engine balance: GpSimd does all f32->f16 casts, ScalarE does xbar issue and
exps, DVE does all PSUM evacuations and reductions.
"""

import numpy as np

import concourse.bass as bass
import concourse.bacc as bacc
import concourse.tile as tile
from concourse import mybir
from concourse.bass_utils import run_bass_kernel_spmd

B, T, H, E = 8, 2048, 512, 512
P = 128
NT = T // P      # 16 tiles along t / s
NH = H // P      # 4 tiles along h
FD = 512         # matmul moving free dim (one fp32 PSUM bank)
NC_T = T // FD   # 4 free-dim chunks along t

F32 = mybir.dt.float32
F16 = mybir.dt.float16
AX = mybir.AxisListType
AF = mybir.ActivationFunctionType

N_CORES = 8

NWARM = 80       # junk warm-up matmuls (N=256) bridging the input-DMA window


def build():
    nc = bacc.Bacc()
    tok_d = nc.declare_dram_parameter("tokens", [T, H], F32, isOutput=False)
    wq_d = nc.declare_dram_parameter("Wq", [H, E], F32, isOutput=False)
    wk_d = nc.declare_dram_parameter("Wk", [H, E], F32, isOutput=False)
    wv_d = nc.declare_dram_parameter("Wv", [H, H], F32, isOutput=False)
    out_d = nc.declare_dram_parameter("out", [T, H], F32, isOutput=True)

    tok_tiled = tok_d.rearrange("(tt p) h -> tt p h", p=P)
    out_tiled = out_d.rearrange("(tt p) h -> tt p h", p=P)

    with tile.TileContext(nc) as tc:
        with (
            tc.tile_pool(name="persist", bufs=1) as pp,
            tc.tile_pool(name="stage", bufs=2) as sp,
            tc.tile_pool(name="ostage", bufs=3) as osp,
            tc.tile_pool(name="stats", bufs=4) as stp,
            tc.tile_pool(name="psum", bufs=2, space=bass.MemorySpace.PSUM) as psp,
        ):
            # ---- warm-up: junk matmuls with no input dependency keep the
            # PE busy (HAM K=8/8) while the input DMAs land.
            junk16 = pp.tile([P, FD], F16, tag="junk16")
            nc.gpsimd.memset(junk16[:], 0)
            junk_ps = psp.tile([P, NC_T * FD], F32, tag="mm", name="junk_ps")
            for i in range(NWARM):
                nc.tensor.matmul(
                    junk_ps[:, (i % 4) * FD : (i % 4) * FD + 256],
                    junk16[:, 0:P],
                    junk16[:, 0:256],
                    start=True,
                    stop=True,
                )
            # Preload the exp ACT table set while ScalarE is idle.
            djunk = stp.tile([P, 1], F32, tag="djunk")
            nc.vector.memset(djunk[:], 0.0)
            dsink = stp.tile([P, 1], F32, tag="dsink")
            nc.scalar.activation(dsink[:], djunk[:], AF.Exp)

            # ---- input DMAs on the sync ring, in arrival-priority order;
            # none has waits, so the ring stays a pure FIFO of input bytes.
            # wk before wq: the Wqk accumulation (hb-major) needs all of
            # WkT but only wq chunk hb, so it chases the wq arrivals.
            # Token tiles 0-3 right after (they gate GT chunk 0), wv next
            # (needed by the first V group), rest of the tokens last.
            wstages = {}
            for name, wd in (("wk", wk_d), ("wq", wq_d)):
                wtiled = wd.rearrange("(hh p) e -> hh p e", p=P)
                for hh in range(NH):
                    ws = sp.tile([P, E], F32, tag="wstage", bufs=8,
                                 name=f"wst_{name}{hh}")
                    nc.sync.dma_start(ws[:], wtiled[hh])
                    wstages[(name, hh)] = ws
            tstages = []
            for tt in range(NT):
                ts = sp.tile([P, H], F32, tag="tstage", bufs=16, name=f"tst{tt}")
                tstages.append(ts)
            for tt in range(4):
                nc.sync.dma_start(tstages[tt][:], tok_tiled[tt])
            wv_stage = sp.tile([P, NH, E], F32, tag="wvstage", bufs=1)
            nc.sync.dma_start(wv_stage[:], wv_d.rearrange("(hh p) e -> p hh e", p=P))
            for tt in range(4, NT):
                nc.sync.dma_start(tstages[tt][:], tok_tiled[tt])

            # ---- casts (GpSimd) + xbar transposes (scalar HWDGE ring) ----
            # wqT/wkT: [e%128, eb, h] fp16
            wT = {}
            for name in ("wq", "wk"):
                wT[name] = pp.tile([P, NH, E], F16, tag=f"{name}T",
                                   name=f"wT_{name}")
            w16s = {}
            for name in ("wk", "wq"):
                for hh in range(NH):
                    w16 = sp.tile([P, E], F16, tag="w16", bufs=8,
                                  name=f"w16_{name}{hh}")
                    nc.gpsimd.tensor_copy(w16[:], wstages[(name, hh)][:])
                    w16s[(name, hh)] = w16
            for name in ("wk", "wq"):
                for hh in range(NH):
                    nc.scalar.dma_start(
                        wT[name][:, :, hh * P : (hh + 1) * P],
                        w16s[(name, hh)][:],
                        transpose=True,
                    )

            # tokens: cast then one xbar per tile -> tokT [h%128, hb, t]
            tokT = pp.tile([P, NH, T], F16, tag="tokT")
            wv16 = pp.tile([P, NH, E], F16, tag="wv16")

            def emit_tok(tt):
                t16 = sp.tile([P, H], F16, tag="t16", bufs=8, name=f"t16_{tt}")
                nc.gpsimd.tensor_copy(t16[:], tstages[tt][:])
                nc.scalar.dma_start(
                    tokT[:, :, tt * P : (tt + 1) * P], t16[:], transpose=True
                )

            for tt in range(4):
                emit_tok(tt)
            for hh in range(NH):
                nc.gpsimd.tensor_copy(wv16[:, hh], wv_stage[:, hh])
            for tt in range(4, NT):
                emit_tok(tt)

            # ---- Wqk = Wq @ Wk.T : [h%128, hb, h'] fp16 ----
            Wqk = pp.tile([P, NH, H], F16, tag="Wqk")
            wqk_ps = psp.tile([P, NC_T * FD], F32, tag="mm", name="wqk_ps")
            for hb in range(NH):
                for eb in range(NH):
                    nc.tensor.matmul(
                        wqk_ps[:, hb * FD : (hb + 1) * FD],
                        wT["wq"][:, eb, hb * P : (hb + 1) * P],
                        wT["wk"][:, eb, :],
                        start=(eb == 0),
                        stop=(eb == NH - 1),
                    )
                nc.vector.tensor_copy(
                    Wqk[:, hb, :], wqk_ps[:, hb * FD : (hb + 1) * FD]
                )

            # ---- GT chunks and V tiles, interleaved per 512-col group ----
            GT = pp.tile([P, NH, T], F16, tag="GT")
            V = pp.tile([P, NT, H], F16, tag="V")
            for sg in range(NC_T):
                gps = psp.tile([P, NC_T * FD], F32, tag="mm", name=f"gps{sg}")
                for gb in range(NH):
                    for hb in range(NH):
                        nc.tensor.matmul(
                            gps[:, gb * FD : (gb + 1) * FD],
                            Wqk[:, hb, gb * P : (gb + 1) * P],
                            tokT[:, hb, sg * FD : (sg + 1) * FD],
                            start=(hb == 0),
                            stop=(hb == NH - 1),
                        )
                    nc.vector.tensor_copy(
                        GT[:, gb, sg * FD : (sg + 1) * FD],
                        gps[:, gb * FD : (gb + 1) * FD],
                    )
                vps = psp.tile([P, NC_T * FD], F32, tag="mm", name=f"vps{sg}")
                for j in range(NC_T):
                    st = sg * NC_T + j
                    for ht in range(NH):
                        nc.tensor.matmul(
                            vps[:, j * FD : (j + 1) * FD],
                            tokT[:, ht, st * P : (st + 1) * P],
                            wv16[:, ht, :],
                            start=(ht == 0),
                            stop=(ht == NH - 1),
                        )
                    nc.vector.tensor_copy(V[:, st, :], vps[:, j * FD : (j + 1) * FD])

            # ---- scores S[s,t] + softmax over t (free axis) ----
            Etile = pp.tile([P, NT, T], F16, tag="E")
            for st in range(NT):
                sps = psp.tile([P, NC_T * FD], F32, tag="mm", name=f"sps{st}")
                for tch in range(NC_T):
                    for hb in range(NH):
                        nc.tensor.matmul(
                            sps[:, tch * FD : (tch + 1) * FD],
                            tokT[:, hb, st * P : (st + 1) * P],
                            GT[:, hb, tch * FD : (tch + 1) * FD],
                            start=(hb == 0),
                            stop=(hb == NH - 1),
                        )
                nmx = stp.tile([P, 1], F32, tag="nmx")
                nc.vector.reduce_max(nmx[:], sps[:], axis=AX.X, negate=True)
                rsum = stp.tile([P, 1], F32, tag="rsum")
                nc.scalar.activation(
                    Etile[:, st, :], sps[:], AF.Exp, bias=nmx[:], accum_out=rsum[:]
                )
                rinv = stp.tile([P, 1], F32, tag="rinv")
                nc.vector.reciprocal(rinv[:], rsum[:])
                # Fold 1/rowsum into V rows (rowsum is per-s, V is s-major).
                nc.vector.tensor_scalar_mul(V[:, st, :], V[:, st, :], rinv[:])

            # ---- context: ctx[t,h] = sum_s E[s,t] * V'[s,h] ----
            for cg in range(NC_T):
                cps = psp.tile([P, NC_T * FD], F32, tag="mm", name=f"cps{cg}")
                for j in range(NC_T):
                    tt = cg * NC_T + j
                    for st in range(NT):
                        nc.tensor.matmul(
                            cps[:, j * FD : (j + 1) * FD],
                            Etile[:, st, tt * P : (tt + 1) * P],
                            V[:, st, :],
                            start=(st == 0),
                            stop=(st == NT - 1),
                        )
                    ot = osp.tile([P, H], F32, tag="ostage", name=f"ost{tt}")
                    if tt >= NT - 2:
                        for cc in range(4):
                            sl = slice(cc * (H // 4), (cc + 1) * (H // 4))
                            nc.vector.tensor_copy(
                                ot[:, sl],
                                cps[:, j * FD + cc * (H // 4) :
                                       j * FD + (cc + 1) * (H // 4)],
                            )
                            nc.sync.dma_start(out_tiled[tt][:, sl], ot[:, sl])
                    else:
                        nc.vector.tensor_copy(ot[:], cps[:, j * FD : (j + 1) * FD])
                        nc.sync.dma_start(out_tiled[tt], ot[:])

    nc.compile()
    return nc


_NC = None


def _get_nc():
    global _NC
    if _NC is None:
        _NC = build()
    return _NC


def _run(inputs, trace=False, **kwargs):
    tokens = np.ascontiguousarray(inputs["tokens"], dtype=np.float32)
    Wq = np.ascontiguousarray(inputs["Wq"], dtype=np.float32)
    Wk = np.ascontiguousarray(inputs["Wk"], dtype=np.float32)
    Wv = np.ascontiguousarray(inputs["Wv"], dtype=np.float32)
    assert tokens.shape == (B, T, H)
    nc = _get_nc()
    in_maps = [
        {"tokens": tokens[i], "Wq": Wq, "Wk": Wk, "Wv": Wv} for i in range(N_CORES)
    ]
    res = run_bass_kernel_spmd(
        nc, in_maps, core_ids=list(range(N_CORES)), trace=trace, **kwargs
    )
    out = np.stack([res.results[i]["out"] for i in range(N_CORES)], axis=0)
    return out.astype(np.float32), res


def kernel(**inputs) -> np.ndarray:
    out, _ = _run(inputs)
    return out


# revision 8
# speedup vs baseline: 1.2810x; 1.2810x over previous
"""Trainium2 Bass kernel for batched attention with softmax over the query axis.

Reference computation (per batch element b):
    Q = tokens @ Wq; K = tokens @ Wk; V = tokens @ Wv
    S = Q @ K.T                [T(t), T(s)]
    A = softmax(S, axis=t)     (normalizes over the *query* axis per key column)
    out = A @ V                [T, H]

Sharding: pure data parallelism - B=8 batch elements, one per NeuronCore.
The softmax couples queries only within a batch element, so no collectives.

Per-core implementation (fp16 matmul operands, fp32 PSUM accumulation):
  - W_qk = Wq @ Wk.T is built on-chip (weight-only work that overlaps the
    token DMA), so scores need one projection G = tokens @ W_qk instead of
    separate Q and K: S = G @ tokens.T.
  - tokT [h%128, h//128, t] = tokens.T via fp16 PE transpose off the DMA
    stage (DVE casts f32->f16 first).
  - GT   [g%128, g//128, t] via lhsT=W_qk rhs=tokT.
  - V    [s%128, s//128, h] via lhsT=tokT rhs=wv16.
  - S_st [s%128, t] accumulates into ONE 4-bank PSUM tile [128, 2048];
    softmax over t (free axis): one DVE reduce_max (negated) and one
    ScalarE exp over all 2048 columns with accum_out row sums.
  - 1/rowsum is folded into V rows (DVE), so the unnormalized exp tile E
    feeds the context matmul, accumulated 4 output tiles per PSUM tile.
  - A short junk-matmul warm-up at t~6us plus the dense W-prep stream keeps
    the PE HAM clock at K=8/8; a dummy exp preloads the ACT table set.
Engine balance: DVE does casts, PSUM evacuations, reductions and V scaling;
ScalarE does GT evacuations and exps; GpSimd only the junk memset.
"""

import numpy as np

import concourse.bass as bass
import concourse.bacc as bacc
import concourse.tile as tile
from concourse import mybir
from concourse.bass_utils import run_bass_kernel_spmd
from concourse.masks import make_identity

B, T, H, E = 8, 2048, 512, 512
P = 128
NT = T // P      # 16 tiles along t / s
NH = H // P      # 4 tiles along h
FD = 512         # matmul moving free dim (one fp32 PSUM bank)
NC_T = T // FD   # 4 free-dim chunks along t
NST = T // FD    # 4 token stage groups (4 t-tiles each)

F32 = mybir.dt.float32
F16 = mybir.dt.float16
AX = mybir.AxisListType
AF = mybir.ActivationFunctionType

N_CORES = 8

NWARM = 16       # junk warm-up matmuls (N=128) before the first W transpose


def build():
    nc = bacc.Bacc()
    tok_d = nc.declare_dram_parameter("tokens", [T, H], F32, isOutput=False)
    wq_d = nc.declare_dram_parameter("Wq", [H, E], F32, isOutput=False)
    wk_d = nc.declare_dram_parameter("Wk", [H, E], F32, isOutput=False)
    wv_d = nc.declare_dram_parameter("Wv", [H, H], F32, isOutput=False)
    out_d = nc.declare_dram_parameter("out", [T, H], F32, isOutput=True)

    # [p, tt, h]: partition = t%128, stage groups of 4 t-tiles -> 1MB DMAs
    tok_staged = tok_d.rearrange("(sg tt p) h -> sg p tt h", p=P, tt=NT // NST)
    out_tiled = out_d.rearrange("(tt p) h -> tt p h", p=P)

    with tile.TileContext(nc) as tc:
        with (
            tc.tile_pool(name="persist", bufs=1) as pp,
            tc.tile_pool(name="stage", bufs=2) as sp,
            tc.tile_pool(name="ostage", bufs=3) as osp,
            tc.tile_pool(name="stats", bufs=4) as stp,
            tc.tile_pool(name="psum", bufs=2, space=bass.MemorySpace.PSUM) as psp,
        ):
            ident32 = pp.tile([P, P], F32, tag="ident32")
            make_identity(nc, ident32[:])
            ident = pp.tile([P, P], F16, tag="ident")
            make_identity(nc, ident[:])

            # Warm-up: junk matmuls (no DMA dependency) so the PE HAM clock
            # is already ramping when the first weight chunk lands.
            junk_ps = psp.tile([P, NC_T * FD], F32, tag="mm", name="junk_ps")
            for i in range(NWARM):
                nc.tensor.matmul(
                    junk_ps[:, (i % 4) * FD : (i % 4) * FD + P],
                    ident[:],
                    ident[:],
                    start=True,
                    stop=True,
                )
            # Preload the exp ACT table set while ScalarE is idle (scalar
            # copies are in every set, so no reload later).
            djunk = stp.tile([P, 1], F32, tag="djunk")
            nc.vector.memset(djunk[:], 0.0)
            dsink = stp.tile([P, 1], F32, tag="dsink")
            nc.scalar.activation(dsink[:], djunk[:], AF.Exp)

            # ---- Wq/Wk: chunked loads, f32 PE transpose straight off the
            # stage, evac to fp16 WqT/WkT.
            wT16 = {}
            for name, wd in (("wq", wq_d), ("wk", wk_d)):
                wT = pp.tile([P, NH, E], F16, tag=f"{name}T", name=f"wT_{name}")
                wtiled = wd.rearrange("(hh p) e -> hh p e", p=P)
                for hh in range(NH):
                    wstage = sp.tile([P, E], F32, tag=f"wstage_{name}",
                                     bufs=4, name=f"wst_{name}{hh}")
                    nc.sync.dma_start(wstage[:], wtiled[hh])
                    for eb in range(NH):
                        ps_tr = psp.tile([P, P], F32, tag="mm",
                                         name=f"tr_{name}{hh}{eb}")
                        nc.tensor.transpose(
                            ps_tr[:], wstage[:, eb * P : (eb + 1) * P], ident32[:]
                        )
                        nc.vector.tensor_copy(
                            wT[:, eb, hh * P : (hh + 1) * P], ps_tr[:]
                        )
                wT16[name] = wT

            # ---- W_qk = Wq @ Wk.T : [h%128, hb, h'] fp16 ----
            Wqk = pp.tile([P, NH, H], F16, tag="Wqk")
            wqk_ps = psp.tile([P, NC_T * FD], F32, tag="mm", name="wqk_ps")
            for hb in range(NH):
                for eb in range(NH):
                    nc.tensor.matmul(
                        wqk_ps[:, hb * FD : (hb + 1) * FD],
                        wT16["wq"][:, eb, hb * P : (hb + 1) * P],
                        wT16["wk"][:, eb, :],
                        start=(eb == 0),
                        stop=(eb == NH - 1),
                    )
                nc.vector.tensor_copy(
                    Wqk[:, hb, :], wqk_ps[:, hb * FD : (hb + 1) * FD]
                )

            # ---- tokens: 1MB staged DMAs; DVE f32->f16 cast then fp16 PE
            # transpose. Per stage group, emit in data-arrival order.
            tokT = pp.tile([P, NH, T], F16, tag="tokT")
            tstages = []
            for sg in range(NST):
                tstage = sp.tile([P, NT // NST, H], F32, tag="tstage", bufs=4,
                                 name=f"tst{sg}")
                tstages.append(tstage)

            for ti in range(NT // NST):
                nc.sync.dma_start(tstages[0][:, ti], tok_staged[0][:, ti])

            # Wv after token stage 0: load f32, cast fp16 on DVE.
            wv_stage = sp.tile([P, NH, E], F32, tag="wvstage", bufs=1)
            nc.sync.dma_start(wv_stage[:], wv_d.rearrange("(hh p) e -> p hh e", p=P))
            wv16 = pp.tile([P, NH, E], F16, tag="wv16")
            for hh in range(NH):
                nc.vector.tensor_copy(wv16[:, hh], wv_stage[:, hh])

            for sg in range(1, NST):
                nc.sync.dma_start(tstages[sg][:], tok_staged[sg])

            def emit_transposes(sg):
                for ti in range(NT // NST):
                    tt = sg * (NT // NST) + ti
                    t16 = sp.tile([P, H], F16, tag="t16", bufs=4, name=f"t16_{tt}")
                    nc.vector.tensor_copy(t16[:], tstages[sg][:, ti])
                    ps_tr = psp.tile([P, NH, P], F16, tag="mm", name=f"trt{tt}")
                    for ht in range(NH):
                        nc.tensor.transpose(
                            ps_tr[:, ht],
                            t16[:, ht * P : (ht + 1) * P],
                            ident[:],
                        )
                    nc.vector.tensor_copy(
                        tokT[:, :, tt * P : (tt + 1) * P], ps_tr[:]
                    )

            # ---- per stage: transposes -> GT chunk -> V tiles ----
            GT = pp.tile([P, NH, T], F16, tag="GT")
            V = pp.tile([P, NT, H], F16, tag="V")
            for sg in range(NST):
                emit_transposes(sg)
                tch = sg
                gps = psp.tile([P, NC_T * FD], F32, tag="mm", name=f"gps{sg}")
                for gb in range(NH):
                    for hb in range(NH):
                        nc.tensor.matmul(
                            gps[:, gb * FD : (gb + 1) * FD],
                            Wqk[:, hb, gb * P : (gb + 1) * P],
                            tokT[:, hb, tch * FD : (tch + 1) * FD],
                            start=(hb == 0),
                            stop=(hb == NH - 1),
                        )
                    nc.scalar.copy(
                        GT[:, gb, tch * FD : (tch + 1) * FD],
                        gps[:, gb * FD : (gb + 1) * FD],
                    )
                vps = psp.tile([P, NC_T * FD], F32, tag="mm", name=f"vps{sg}")
                for j in range(NC_T):
                    st = sg * NC_T + j
                    for ht in range(NH):
                        nc.tensor.matmul(
                            vps[:, j * FD : (j + 1) * FD],
                            tokT[:, ht, st * P : (st + 1) * P],
                            wv16[:, ht, :],
                            start=(ht == 0),
                            stop=(ht == NH - 1),
                        )
                    nc.vector.tensor_copy(V[:, st, :], vps[:, j * FD : (j + 1) * FD])

            # ---- scores S[s,t] + softmax over t (free axis) ----
            Etile = pp.tile([P, NT, T], F16, tag="E")
            for st in range(NT):
                sps = psp.tile([P, NC_T * FD], F32, tag="mm", name=f"sps{st}")
                for tch in range(NC_T):
                    for hb in range(NH):
                        nc.tensor.matmul(
                            sps[:, tch * FD : (tch + 1) * FD],
                            tokT[:, hb, st * P : (st + 1) * P],
                            GT[:, hb, tch * FD : (tch + 1) * FD],
                            start=(hb == 0),
                            stop=(hb == NH - 1),
                        )
                nmx = stp.tile([P, 1], F32, tag="nmx")
                nc.vector.reduce_max(nmx[:], sps[:], axis=AX.X, negate=True)
                rsum = stp.tile([P, 1], F32, tag="rsum")
                nc.scalar.activation(
                    Etile[:, st, :], sps[:], AF.Exp, bias=nmx[:], accum_out=rsum[:]
                )
                rinv = stp.tile([P, 1], F32, tag="rinv")
                nc.vector.reciprocal(rinv[:], rsum[:])
                # Fold 1/rowsum into V rows (rowsum is per-s, V is s-major).
                nc.vector.tensor_scalar_mul(V[:, st, :], V[:, st, :], rinv[:])

            # ---- context: ctx[t,h] = sum_s E[s,t] * V'[s,h] ----
            for cg in range(NC_T):
                cps = psp.tile([P, NC_T * FD], F32, tag="mm", name=f"cps{cg}")
                for j in range(NC_T):
                    tt = cg * NC_T + j
                    for st in range(NT):
                        nc.tensor.matmul(
                            cps[:, j * FD : (j + 1) * FD],
                            Etile[:, st, tt * P : (tt + 1) * P],
                            V[:, st, :],
                            start=(st == 0),
                            stop=(st == NT - 1),
                        )
                    ot = osp.tile([P, H], F32, tag="ostage", name=f"ost{tt}")
                    if tt >= NT - 2:
                        for cc in range(4):
                            sl = slice(cc * (H // 4), (cc + 1) * (H // 4))
                            nc.vector.tensor_copy(
                                ot[:, sl],
                                cps[:, j * FD + cc * (H // 4) :
                                       j * FD + (cc + 1) * (H // 4)],
                            )
                            nc.sync.dma_start(out_tiled[tt][:, sl], ot[:, sl])
                    else:
                        nc.vector.tensor_copy(ot[:], cps[:, j * FD : (j + 1) * FD])
                        nc.sync.dma_start(out_tiled[tt], ot[:])

    nc.compile()
    return nc


_NC = None


def _get_nc():
    global _NC
    if _NC is None:
        _NC = build()
    return _NC


def _run(inputs, trace=False, **kwargs):
    tokens = np.ascontiguousarray(inputs["tokens"], dtype=np.float32)
    Wq = np.ascontiguousarray(inputs["Wq"], dtype=np.float32)
    Wk = np.ascontiguousarray(inputs["Wk"], dtype=np.float32)
    Wv = np.ascontiguousarray(inputs["Wv"], dtype=np.float32)
    assert tokens.shape == (B, T, H)
    nc = _get_nc()
    in_maps = [
        {"tokens": tokens[i], "Wq": Wq, "Wk": Wk, "Wv": Wv} for i in range(N_CORES)
    ]
    res = run_bass_kernel_spmd(
        nc, in_maps, core_ids=list(range(N_CORES)), trace=trace, **kwargs
    )
    out = np.stack([res.results[i]["out"] for i in range(N_CORES)], axis=0)
    return out.astype(np.float32), res


def kernel(**inputs) -> np.ndarray:
    out, _ = _run(inputs)
    return out


# revision 11
# speedup vs baseline: 1.4478x; 1.1302x over previous
"""Trainium2 Bass kernel for batched attention with softmax over the query axis.

Reference computation (per batch element b):
    Q = tokens @ Wq; K = tokens @ Wk; V = tokens @ Wv
    S = Q @ K.T                [T(t), T(s)]
    A = softmax(S, axis=t)     (normalizes over the *query* axis per key column)
    out = A @ V                [T, H]

Sharding: pure data parallelism - B=8 batch elements, one per NeuronCore.
The softmax couples queries only within a batch element, so no collectives.

Per-core implementation (fp16 matmul operands, fp32 PSUM accumulation):
  - W_qk = Wq @ Wk.T is built on-chip so scores need one projection
    G = tokens @ W_qk instead of separate Q and K: S = G @ tokens.T.
  - ALL transposes are PLAIN matmuls against a stationary data tile with a
    streaming fp16 identity (out = X.T @ I). Unlike transpose-mode matmuls,
    plain matmuls get the LDWEIGHTS background-buffer pull-ahead, so
    back-to-back 128x128 transposes pace at ~80ns instead of ~215ns.
  - Weights are DVE-cast to fp16 before transposing (fp16 LDW is 2x fp32).
  - DMA order wk -> token stage 0 -> wq -> wv -> stages 1-3 so the two GT
    prerequisites (Wqk and tokT[0:512]) complete at about the same time.
  - tokT tiles for stage sg+1 are emitted after V(sg) so the PE never waits.
  - S tile [s%128, t]: per-chunk PSUM max (overlaps the score matmuls), one
    combined negated max, exp per chunk with accum_out row sums
    (ScalarE), 1/rowsum folded into V rows (DVE).
  - Junk matmuls at t~8us warm the PE HAM clock before the first transpose;
    a dummy exp preloads the ACT exp table set off the critical path.
Engine balance: DVE does casts, PSUM evacuations, reductions, V scaling;
ScalarE does GT evacuations and exps; GpSimd idle.
"""

import numpy as np

import concourse.bass as bass
import concourse.bacc as bacc
import concourse.tile as tile
from concourse import mybir
from concourse.bass_utils import run_bass_kernel_spmd
from concourse.masks import make_identity

B, T, H, E = 8, 2048, 512, 512
P = 128
NT = T // P      # 16 tiles along t / s
NH = H // P      # 4 tiles along h
FD = 512         # matmul moving free dim (one fp32 PSUM bank)
NC_T = T // FD   # 4 free-dim chunks along t
NST = T // FD    # 4 token stage groups (4 t-tiles each)

F32 = mybir.dt.float32
F16 = mybir.dt.float16
AX = mybir.AxisListType
AF = mybir.ActivationFunctionType

N_CORES = 8

NWARM = 20       # junk warm-up matmuls (N=128) before the first W transpose


def build():
    nc = bacc.Bacc()
    tok_d = nc.declare_dram_parameter("tokens", [T, H], F32, isOutput=False)
    wq_d = nc.declare_dram_parameter("Wq", [H, E], F32, isOutput=False)
    wk_d = nc.declare_dram_parameter("Wk", [H, E], F32, isOutput=False)
    wv_d = nc.declare_dram_parameter("Wv", [H, H], F32, isOutput=False)
    out_d = nc.declare_dram_parameter("out", [T, H], F32, isOutput=True)

    tok_staged = tok_d.rearrange("(sg tt p) h -> sg p tt h", p=P, tt=NT // NST)
    out_tiled = out_d.rearrange("(tt p) h -> tt p h", p=P)

    with tile.TileContext(nc) as tc:
        with (
            tc.tile_pool(name="persist", bufs=1) as pp,
            tc.tile_pool(name="stage", bufs=2) as sp,
            tc.tile_pool(name="ostage", bufs=3) as osp,
            tc.tile_pool(name="stats", bufs=4) as stp,
            tc.tile_pool(name="psum", bufs=8, space=bass.MemorySpace.PSUM) as psp,
        ):
            ident = pp.tile([P, P], F16, tag="ident")
            make_identity(nc, ident[:])

            # Warm-up: junk matmuls (no DMA dependency) so the PE HAM clock
            # is ramping by the time the first weight chunk lands.
            junk_ps = psp.tile([P, FD], F32, tag="mm", name="junk_ps")
            for i in range(NWARM):
                nc.tensor.matmul(
                    junk_ps[:, 0:P], ident[:], ident[:], start=True, stop=True
                )
            # Preload the exp ACT table set while ScalarE is idle (plain
            # copies exist in every set, so no reload later).
            djunk = stp.tile([P, 1], F32, tag="djunk")
            nc.vector.memset(djunk[:], 0.0)
            dsink = stp.tile([P, 1], F32, tag="dsink")
            nc.scalar.activation(dsink[:], djunk[:], AF.Exp)

            # ---- input DMAs: wk chunks, token stage 0 tiles, wq chunks,
            # wv, token stages 1-3 ----
            wstages = {}
            for name, wd in (("wk", wk_d), ("wq", wq_d)):
                wtiled = wd.rearrange("(hh p) e -> hh p e", p=P)
                for hh in range(NH):
                    ws = sp.tile([P, E], F32, tag="wstage", bufs=8,
                                 name=f"wst_{name}{hh}")
                    if name == "wk":
                        nc.sync.dma_start(ws[:], wtiled[hh])
                    wstages[(name, hh)] = ws

            tstages = []
            for sg in range(NST):
                tstage = sp.tile([P, NT // NST, H], F32, tag="tstage", bufs=4,
                                 name=f"tst{sg}")
                tstages.append(tstage)
            for ti in range(NT // NST):
                nc.sync.dma_start(tstages[0][:, ti], tok_staged[0][:, ti])

            wq_tiled = wq_d.rearrange("(hh p) e -> hh p e", p=P)
            for hh in range(NH):
                nc.sync.dma_start(wstages[("wq", hh)][:], wq_tiled[hh])

            wv_stage = sp.tile([P, NH, E], F32, tag="wvstage", bufs=1)
            nc.sync.dma_start(wv_stage[:], wv_d.rearrange("(hh p) e -> p hh e", p=P))
            for sg in range(1, NST):
                nc.sync.dma_start(tstages[sg][:], tok_staged[sg])

            # ---- W transposes (plain matmul vs identity) ----
            wT16 = {}
            w16s = {}
            for name in ("wk", "wq"):
                wT = pp.tile([P, NH, E], F16, tag=f"{name}T", name=f"wT_{name}")
                wT16[name] = wT

            def emit_wT(name):
                # DVE cast f32->f16, then 16 plain-matmul transposes.
                for hh in range(NH):
                    w16 = sp.tile([P, E], F16, tag="w16", bufs=8,
                                  name=f"w16_{name}{hh}")
                    nc.vector.tensor_copy(w16[:], wstages[(name, hh)][:])
                    w16s[(name, hh)] = w16
                for hh in range(NH):
                    for eb in range(NH):
                        ps_tr = psp.tile([P, P], F32, tag="mm",
                                         name=f"tr_{name}{hh}{eb}")
                        nc.tensor.matmul(
                            ps_tr[:],
                            w16s[(name, hh)][:, eb * P : (eb + 1) * P],
                            ident[:],
                            start=True,
                            stop=True,
                        )
                        nc.vector.tensor_copy(
                            wT16[name][:, eb, hh * P : (hh + 1) * P], ps_tr[:]
                        )

            emit_wT("wk")

            # ---- token transposes for one stage group ----
            tokT = pp.tile([P, NH, T], F16, tag="tokT")

            def emit_transposes(sg):
                for ti in range(NT // NST):
                    tt = sg * (NT // NST) + ti
                    t16 = sp.tile([P, H], F16, tag="t16", bufs=4, name=f"t16_{tt}")
                    nc.vector.tensor_copy(t16[:], tstages[sg][:, ti])
                    ps_tr = psp.tile([P, NH, P], F32, tag="mm", name=f"trt{tt}")
                    for ht in range(NH):
                        nc.tensor.matmul(
                            ps_tr[:, ht],
                            t16[:, ht * P : (ht + 1) * P],
                            ident[:],
                            start=True,
                            stop=True,
                        )
                    nc.vector.tensor_copy(
                        tokT[:, :, tt * P : (tt + 1) * P], ps_tr[:]
                    )

            emit_transposes(0)
            emit_wT("wq")
            wv16 = pp.tile([P, NH, E], F16, tag="wv16")
            for hh in range(NH):
                nc.vector.tensor_copy(wv16[:, hh], wv_stage[:, hh])

            # ---- W_qk = Wq @ Wk.T : [h%128, hb, h'] fp16 ----
            Wqk = pp.tile([P, NH, H], F16, tag="Wqk")
            for hb in range(NH):
                ps = psp.tile([P, FD], F32, tag="mm", name=f"ps_wqk{hb}")
                for eb in range(NH):
                    nc.tensor.matmul(
                        ps[:],
                        wT16["wq"][:, eb, hb * P : (hb + 1) * P],
                        wT16["wk"][:, eb, :],
                        start=(eb == 0),
                        stop=(eb == NH - 1),
                    )
                nc.vector.tensor_copy(Wqk[:, hb, :], ps[:])

            # ---- per stage: GT chunk -> V tiles -> next stage transposes ----
            GT = pp.tile([P, NH, T], F16, tag="GT")
            V = pp.tile([P, NT, H], F16, tag="V")
            for sg in range(NST):
                tch = sg
                for gb in range(NH):
                    ps = psp.tile([P, FD], F32, tag="mm", name=f"ps_g{gb}_{tch}")
                    for hb in range(NH):
                        nc.tensor.matmul(
                            ps[:],
                            Wqk[:, hb, gb * P : (gb + 1) * P],
                            tokT[:, hb, tch * FD : (tch + 1) * FD],
                            start=(hb == 0),
                            stop=(hb == NH - 1),
                        )
                    nc.scalar.copy(GT[:, gb, tch * FD : (tch + 1) * FD], ps[:])
                for st in range(sg * NC_T, (sg + 1) * NC_T):
                    ps = psp.tile([P, FD], F32, tag="mm", name=f"ps_v{st}")
                    for ht in range(NH):
                        nc.tensor.matmul(
                            ps[:],
                            tokT[:, ht, st * P : (st + 1) * P],
                            wv16[:, ht, :],
                            start=(ht == 0),
                            stop=(ht == NH - 1),
                        )
                    nc.vector.tensor_copy(V[:, st, :], ps[:])
                if sg + 1 < NST:
                    emit_transposes(sg + 1)

            # ---- scores S[s,t] + softmax over t (free axis) ----
            Etile = pp.tile([P, NT, T], F16, tag="E")
            for st in range(NT):
                pss = [
                    psp.tile([P, FD], F32, tag="mm", name=f"ps_s{st}_{tch}")
                    for tch in range(NC_T)
                ]
                mx4 = stp.tile([P, NC_T], F32, tag="mx4")
                for tch in range(NC_T):
                    for hb in range(NH):
                        nc.tensor.matmul(
                            pss[tch][:],
                            tokT[:, hb, st * P : (st + 1) * P],
                            GT[:, hb, tch * FD : (tch + 1) * FD],
                            start=(hb == 0),
                            stop=(hb == NH - 1),
                        )
                    nc.vector.reduce_max(
                        mx4[:, tch : tch + 1], pss[tch][:], axis=AX.X
                    )
                nmx = stp.tile([P, 1], F32, tag="nmx")
                nc.vector.reduce_max(nmx[:], mx4[:], axis=AX.X, negate=True)
                racc = stp.tile([P, NC_T], F32, tag="racc")
                for tch in range(NC_T):
                    nc.scalar.activation(
                        Etile[:, st, tch * FD : (tch + 1) * FD],
                        pss[tch][:],
                        AF.Exp,
                        bias=nmx[:],
                        accum_out=racc[:, tch : tch + 1],
                    )
                rsum = stp.tile([P, 1], F32, tag="rsum")
                nc.vector.reduce_sum(rsum[:], racc[:], axis=AX.X)
                rinv = stp.tile([P, 1], F32, tag="rinv")
                nc.vector.reciprocal(rinv[:], rsum[:])
                # Fold 1/rowsum into V rows (rowsum is per-s, V is s-major).
                nc.vector.tensor_scalar_mul(V[:, st, :], V[:, st, :], rinv[:])

            # ---- context: ctx[t,h] = sum_s E[s,t] * V'[s,h] ----
            for tt in range(NT):
                ps = psp.tile([P, FD], F32, tag="mm", name=f"ps_c{tt}")
                for st in range(NT):
                    nc.tensor.matmul(
                        ps[:],
                        Etile[:, st, tt * P : (tt + 1) * P],
                        V[:, st, :],
                        start=(st == 0),
                        stop=(st == NT - 1),
                    )
                ot = osp.tile([P, H], F32, tag="ostage", name=f"ost{tt}")
                if tt >= NT - 2:
                    for cc in range(2):
                        sl = slice(cc * (H // 2), (cc + 1) * (H // 2))
                        nc.vector.tensor_copy(ot[:, sl], ps[:, sl])
                        nc.sync.dma_start(out_tiled[tt][:, sl], ot[:, sl])
                else:
                    nc.vector.tensor_copy(ot[:], ps[:])
                    nc.sync.dma_start(out_tiled[tt], ot[:])

    nc.compile()
    return nc


_NC = None


def _get_nc():
    global _NC
    if _NC is None:
        _NC = build()
    return _NC


def _run(inputs, trace=False, **kwargs):
    tokens = np.ascontiguousarray(inputs["tokens"], dtype=np.float32)
    Wq = np.ascontiguousarray(inputs["Wq"], dtype=np.float32)
    Wk = np.ascontiguousarray(inputs["Wk"], dtype=np.float32)
    Wv = np.ascontiguousarray(inputs["Wv"], dtype=np.float32)
    assert tokens.shape == (B, T, H)
    nc = _get_nc()
    in_maps = [
        {"tokens": tokens[i], "Wq": Wq, "Wk": Wk, "Wv": Wv} for i in range(N_CORES)
    ]
    res = run_bass_kernel_spmd(
        nc, in_maps, core_ids=list(range(N_CORES)), trace=trace, **kwargs
    )
    out = np.stack([res.results[i]["out"] for i in range(N_CORES)], axis=0)
    return out.astype(np.float32), res


def kernel(**inputs) -> np.ndarray:
    out, _ = _run(inputs)
    return out
